# revision 10
# baseline (speedup 1.0000x reference)
"""Liquid State Machine Bass kernel for Trainium2 (8 NeuronCores, SPMD).

Strategy
--------
Data-parallel over batch: B=2048 is sharded 256/core; input_weight and
reservoir_weight are replicated. All device state is kept TRANSPOSED
(neurons on SBUF partitions, batch on the free dim) so the per-step spike
tensor is directly usable as the matmul moving operand with W tiles as
stationary weights -- no on-device transposes at all (the host passes
pre-transposed slices, which is free).

The recurrent matmul rec = spikes @ W runs in float32r mode (PE truncates
operands to FP22 but streams at 1 cycle/row instead of fp32's 4).  W is
split on the host into W_hi + W_lo, both exactly representable in FP22
(13-mantissa-bit chunks of the fp32 value), and spikes are exactly 0/1, so
two f32r matmuls accumulated in fp32 PSUM reproduce the fp32 product to
~2^-27 -- better than a native fp32 matmul, at half its cost.

State is rescaled to fuse the update arithmetic into single DVE ops:
  z = c / beta_syn           ->  z' = a_syn * z + (ic + rec)
  w = v / (beta_syn*beta_mem)->  w' = a_mem * w + z'
with spikes (c>0) == (z>0) and (v>0) == (w>0).  Outputs are rescaled once
at the end.
"""

import numpy as np

import concourse.bass as bass
import concourse.mybir as mybir
import concourse.tile as tile
from concourse import bacc
from concourse.bass_utils import run_bass_kernel_spmd

F32 = mybir.dt.float32
F32R = mybir.dt.float32r
OP = mybir.AluOpType

N_CORES = 8
B, D, N = 2048, 256, 512
BL = B // N_CORES          # batch per core
KT = N // 128              # 4 neuron/contraction tiles
DT = D // 128              # 2 input-dim tiles

TAU_MEM = 20.0
TAU_SYN = 5.0
A_MEM = float(np.exp(-1.0 / TAU_MEM))
A_SYN = float(np.exp(-1.0 / TAU_SYN))
B_MEM = 1.0 - A_MEM
B_SYN = 1.0 - A_SYN


def _rne_m11(x: np.ndarray) -> np.ndarray:
    """Round fp32 to 11 mantissa bits, round-to-nearest-even.

    The TRN2 PE's float32r mode rounds matmul operands to e8m11 RNE
    (measured on hardware; the "FP22" in the docs is not what this silicon
    does).  A Dekker split W = rne_m11(W) + (W - rne_m11(W)) yields two
    parts of <=12 significant bits each, so both pass through the PE
    exactly and two f32r matmuls reproduce the fp32 product.
    """
    xi = x.view(np.uint32)
    r = (xi + np.uint32(0x7FF) + ((xi >> np.uint32(12)) & np.uint32(1))) & np.uint32(
        0xFFFFF000
    )
    return r.view(np.float32)


def _lsm_body(tc, n_steps, io):
    nc = tc.nc
    with tc.tile_pool(name="const", bufs=1) as const, \
         tc.tile_pool(name="state", bufs=1) as state, \
         tc.tile_pool(name="g", bufs=2) as gpool, \
         tc.tile_pool(name="tmp", bufs=3) as tpool, \
         tc.tile_pool(name="outs", bufs=4) as opool, \
         tc.tile_pool(name="ps", bufs=8, space="PSUM") as psum:

        # ---- load constants ------------------------------------------------
        whi = []
        wlo = []
        for k in range(KT):
            th = const.tile([128, N], F32R, tag=f"whi{k}")
            nc.sync.dma_start(th[:], io["w_hi"][k * 128:(k + 1) * 128, :])
            whi.append(th)
            tl = const.tile([128, N], F32R, tag=f"wlo{k}")
            nc.sync.dma_start(tl[:], io["w_lo"][k * 128:(k + 1) * 128, :])
            wlo.append(tl)

        iwt = []
        rt = []
        for d in range(DT):
            ti = const.tile([128, N], F32, tag=f"iwt{d}")
            nc.sync.dma_start(ti[:], io["iwT"][d * 128:(d + 1) * 128, :])
            iwt.append(ti)
            tr = const.tile([128, BL], F32, tag=f"ratesT{d}")
            nc.sync.dma_start(tr[:], io["ratesT"][d * 128:(d + 1) * 128, :])
            rt.append(tr)

        # ---- input current: icT = iw @ rates.T, true-fp32 matmul -----------
        ic = []
        for j in range(KT):
            pt = psum.tile([128, BL], F32, tag="ps")
            for d in range(DT):
                nc.tensor.matmul(
                    pt[:],
                    iwt[d][:, j * 128:(j + 1) * 128],
                    rt[d][:],
                    start=(d == 0),
                    stop=(d == DT - 1),
                )
            ti = const.tile([128, BL], F32, tag=f"ic{j}")
            nc.vector.tensor_copy(ti[:], pt[:])
            ic.append(ti)

        # ---- state init ----------------------------------------------------
        z = []
        w = []
        ssum = []
        g = []
        for j in range(KT):
            sl = slice(j * 128, (j + 1) * 128)
            tz = state.tile([128, BL], F32, tag=f"z{j}")
            nc.sync.dma_start(tz[:], io["c0T"][sl, :])
            nc.vector.tensor_scalar(tz[:], tz[:], 1.0 / B_SYN, None, OP.mult)
            z.append(tz)
            tw = state.tile([128, BL], F32, tag=f"w{j}")
            nc.sync.dma_start(tw[:], io["v0T"][sl, :])
            nc.vector.tensor_scalar(tw[:], tw[:], 1.0 / (B_SYN * B_MEM), None, OP.mult)
            w.append(tw)
            ts_ = state.tile([128, BL], F32, tag=f"ss{j}")
            nc.gpsimd.memset(ts_[:], 0.0)
            ssum.append(ts_)
            tg = gpool.tile([128, BL], F32R, tag=f"g{j}")
            nc.gpsimd.tensor_scalar(tg[:], tz[:], 0.0, None, OP.is_gt)
            g.append(tg)

        # ---- main recurrence (fully unrolled) ------------------------------
        for _t in range(n_steps):
            g_new = [None] * KT
            for j in range(KT):
                jsl = slice(j * 128, (j + 1) * 128)
                pt = psum.tile([128, BL], F32, tag="ps")
                for k in range(KT):
                    nc.tensor.matmul(
                        pt[:],
                        whi[k][:, jsl],
                        g[k][:],
                        start=(k == 0),
                        stop=False,
                    )
                    nc.tensor.matmul(
                        pt[:],
                        wlo[k][:, jsl],
                        g[k][:],
                        start=False,
                        stop=(k == KT - 1),
                    )
                # z_j = a_syn*z_j + (rec + ic_j)
                tt = tpool.tile([128, BL], F32, tag=f"t{j}")
                nc.vector.tensor_tensor(tt[:], pt[:], ic[j][:], OP.add)
                nc.vector.scalar_tensor_tensor(
                    z[j][:], z[j][:], A_SYN, tt[:], OP.mult, OP.add
                )
                # next-step spikes
                tg = gpool.tile([128, BL], F32R, tag=f"g{j}")
                nc.gpsimd.tensor_scalar(tg[:], z[j][:], 0.0, None, OP.is_gt)
                g_new[j] = tg
                # w_j = a_mem*w_j + z_j ; ssum_j += (w_j > 0)
                nc.vector.scalar_tensor_tensor(
                    w[j][:], w[j][:], A_MEM, z[j][:], OP.mult, OP.add
                )
                nc.vector.scalar_tensor_tensor(
                    ssum[j][:], w[j][:], 0.0, ssum[j][:], OP.is_gt, OP.add
                )
            g = g_new

        # ---- outputs -------------------------------------------------------
        for j in range(KT):
            sl = slice(j * 128, (j + 1) * 128)
            ro = opool.tile([128, BL], F32, tag="out")
            nc.vector.tensor_scalar(ro[:], ssum[j][:], 1.0 / n_steps, None, OP.mult)
            nc.sync.dma_start(io["roT"][sl, :], ro[:])
            co = opool.tile([128, BL], F32, tag="out")
            nc.vector.tensor_scalar(co[:], z[j][:], B_SYN, None, OP.mult)
            nc.sync.dma_start(io["cT"][sl, :], co[:])
            vo = opool.tile([128, BL], F32, tag="out")
            nc.vector.tensor_scalar(vo[:], w[j][:], B_SYN * B_MEM, None, OP.mult)
            nc.sync.dma_start(io["vT"][sl, :], vo[:])


def _build(n_steps: int):
    nc = bacc.Bacc("TRN2", target_bir_lowering=False, debug=False)
    io = {
        "ratesT": nc.dram_tensor("ratesT", [D, BL], F32, kind="ExternalInput").ap(),
        "iwT": nc.dram_tensor("iwT", [D, N], F32, kind="ExternalInput").ap(),
        "w_hi": nc.dram_tensor("w_hi", [N, N], F32R, kind="ExternalInput").ap(),
        "w_lo": nc.dram_tensor("w_lo", [N, N], F32R, kind="ExternalInput").ap(),
        "v0T": nc.dram_tensor("v0T", [N, BL], F32, kind="ExternalInput").ap(),
        "c0T": nc.dram_tensor("c0T", [N, BL], F32, kind="ExternalInput").ap(),
        "roT": nc.dram_tensor("roT", [N, BL], F32, kind="ExternalOutput").ap(),
        "vT": nc.dram_tensor("vT", [N, BL], F32, kind="ExternalOutput").ap(),
        "cT": nc.dram_tensor("cT", [N, BL], F32, kind="ExternalOutput").ap(),
    }
    with tile.TileContext(nc) as tc:
        _lsm_body(tc, n_steps, io)
    nc.compile()
    return nc


_NC_CACHE = {}


def _get_nc(n_steps: int):
    if n_steps not in _NC_CACHE:
        _NC_CACHE[n_steps] = _build(n_steps)
    return _NC_CACHE[n_steps]


def make_in_maps(rates, input_weight, reservoir_weight, v_mem, current):
    rates = np.ascontiguousarray(np.asarray(rates, dtype=np.float32))
    iw = np.ascontiguousarray(np.asarray(input_weight, dtype=np.float32))
    W = np.ascontiguousarray(np.asarray(reservoir_weight, dtype=np.float32))
    v0 = np.ascontiguousarray(np.asarray(v_mem, dtype=np.float32))
    c0 = np.ascontiguousarray(np.asarray(current, dtype=np.float32))

    w_hi = _rne_m11(W)
    w_lo = (W - w_hi).astype(np.float32)
    # deep-denormal residuals (<~1e-42, impossible for real data) are not
    # m11-exact; flushing them loses nothing measurable
    w_lo = np.where(_rne_m11(w_lo) == w_lo, w_lo, np.float32(0.0))
    w_hi = np.ascontiguousarray(w_hi)
    w_lo = np.ascontiguousarray(w_lo)
    iwT = np.ascontiguousarray(iw.T)

    in_maps = []
    for i in range(N_CORES):
        sl = slice(i * BL, (i + 1) * BL)
        in_maps.append({
            "ratesT": np.ascontiguousarray(rates[sl].T),
            "iwT": iwT,
            "w_hi": w_hi,
            "w_lo": w_lo,
            "v0T": np.ascontiguousarray(v0[sl].T),
            "c0T": np.ascontiguousarray(c0[sl].T),
        })
    return in_maps


def assemble(results):
    readout = np.empty((B, N), np.float32)
    v = np.empty((B, N), np.float32)
    c = np.empty((B, N), np.float32)
    for i, res in enumerate(results):
        sl = slice(i * BL, (i + 1) * BL)
        readout[sl] = res["roT"].T
        v[sl] = res["vT"].T
        c[sl] = res["cT"].T
    return readout, (v, c)


def kernel(rates, input_weight, reservoir_weight, v_mem, current, n_steps,
           _trace=False, _trace_kwargs=None):
    n_steps = int(n_steps)
    nc = _get_nc(n_steps)
    in_maps = make_in_maps(rates, input_weight, reservoir_weight, v_mem, current)
    res = run_bass_kernel_spmd(
        nc, in_maps, core_ids=list(range(N_CORES)),
        trace=_trace, **(_trace_kwargs or {}),
    )
    out = assemble(res.results)
    if _trace:
        kernel.last_result = res
    return out


# revision 11
# speedup vs baseline: 3.3417x; 3.3417x over previous
"""Liquid State Machine Bass kernel for Trainium2 (8 NeuronCores, SPMD).

Strategy
--------
Data-parallel over batch: B=2048 is sharded 256/core; input_weight and
reservoir_weight are replicated. All device state is kept TRANSPOSED
(neurons on SBUF partitions, batch on the free dim) so the per-step spike
tensor is directly usable as the matmul moving operand with W tiles as
stationary weights -- no on-device transposes (the host passes
pre-transposed slices, which is free).

The recurrent matmul rec = spikes @ W runs in float32r mode: the PE rounds
operands to e8m11 (measured on HW) but streams at 1 cycle/row vs fp32's 4.
W is Dekker-split on the host into W_hi + W_lo (12 significant bits each,
both exact under the PE's m11 rounding), and spikes are exactly 0/1, so two
f32r matmuls accumulated in fp32 PSUM reproduce the fp32 product.

The per-step input current is folded into the same PSUM accumulation as
two identity-stationary f32r matmuls of an m11-split of ic (split once on
device), which removes a vector-engine op from the inner loop.  State is
rescaled (z = c/beta_syn, w = v/(beta_syn*beta_mem)) so each state update
is a single scalar_tensor_tensor op, and neuron tiles are processed in
pairs (128x512 DVE ops) to amortize per-instruction overhead.  GpSimd is
avoided in the loop (its elementwise ops are ~4us each on HW).
"""

import numpy as np

import concourse.bass as bass
import concourse.mybir as mybir
import concourse.tile as tile
from concourse import bacc
from concourse.bass_utils import run_bass_kernel_spmd
from concourse.masks import make_identity

F32 = mybir.dt.float32
F32R = mybir.dt.float32r
U32 = mybir.dt.uint32
OP = mybir.AluOpType

N_CORES = 8
B, D, N = 2048, 256, 512
BL = B // N_CORES          # batch per core
KT = N // 128              # 4 neuron/contraction tiles
DT = D // 128              # 2 input-dim tiles
NP = KT // 2               # 2 tile-pairs

TAU_MEM = 20.0
TAU_SYN = 5.0
A_MEM = float(np.exp(-1.0 / TAU_MEM))
A_SYN = float(np.exp(-1.0 / TAU_SYN))
B_MEM = 1.0 - A_MEM
B_SYN = 1.0 - A_SYN


def _rne_m11(x: np.ndarray) -> np.ndarray:
    """Round fp32 to 11 mantissa bits, round-to-nearest-even (the PE's
    float32r operand rounding, measured on TRN2 hardware)."""
    xi = x.view(np.uint32)
    r = (xi + np.uint32(0x7FF) + ((xi >> np.uint32(12)) & np.uint32(1))) & np.uint32(
        0xFFFFF000
    )
    return r.view(np.float32)


def _lsm_body(tc, n_steps, io):
    nc = tc.nc
    with tc.tile_pool(name="const", bufs=1) as const, \
         tc.tile_pool(name="state", bufs=1) as state, \
         tc.tile_pool(name="g", bufs=2) as gpool, \
         tc.tile_pool(name="outs", bufs=4) as opool, \
         tc.tile_pool(name="ps", bufs=4, space="PSUM") as psum:

        # ---- load constants ------------------------------------------------
        whi = []
        wlo = []
        for k in range(KT):
            th = const.tile([128, N], F32R, tag=f"whi{k}")
            nc.sync.dma_start(th[:], io["w_hi"][k * 128:(k + 1) * 128, :])
            whi.append(th)
            tl = const.tile([128, N], F32R, tag=f"wlo{k}")
            nc.sync.dma_start(tl[:], io["w_lo"][k * 128:(k + 1) * 128, :])
            wlo.append(tl)

        iwt = []
        rt = []
        for d in range(DT):
            ti = const.tile([128, N], F32, tag=f"iwt{d}")
            nc.sync.dma_start(ti[:], io["iwT"][d * 128:(d + 1) * 128, :])
            iwt.append(ti)
            tr = const.tile([128, BL], F32, tag=f"ratesT{d}")
            nc.sync.dma_start(tr[:], io["ratesT"][d * 128:(d + 1) * 128, :])
            rt.append(tr)

        ident = const.tile([128, 128], F32, tag="identf")
        make_identity(nc, ident[:])
        identr = const.tile([128, 128], F32R, tag="identr")
        nc.vector.tensor_copy(identr[:], ident[:])

        # ---- input current: icT = iw @ rates.T (true fp32), m11-split ------
        # pair p holds n-tiles (2p, 2p+1) side by side: [128, 512]
        ichi = []
        iclo = []
        for p in range(NP):
            pt = psum.tile([128, 2 * BL], F32, tag="ps")
            for h in range(2):
                j = 2 * p + h
                jsl = slice(j * 128, (j + 1) * 128)
                hsl = slice(h * BL, (h + 1) * BL)
                for d in range(DT):
                    nc.tensor.matmul(
                        pt[:, hsl], iwt[d][:, jsl], rt[d][:],
                        start=(d == 0), stop=(d == DT - 1),
                        skip_group_check=True,
                    )
            icf = const.tile([128, 2 * BL], F32, tag=f"icf{p}")
            nc.vector.tensor_copy(icf[:], pt[:])
            # m11 Dekker split on device (uint32 bit arithmetic)
            icu = icf[:].bitcast(U32)
            tu = const.tile([128, 2 * BL], U32, tag=f"ictmp{p}")
            nc.vector.tensor_scalar(tu[:], icu, 12, None, OP.logical_shift_right)
            nc.vector.tensor_scalar(tu[:], tu[:], 1, None, OP.bitwise_and)
            nc.vector.scalar_tensor_tensor(tu[:], tu[:], 0x7FF, icu, OP.add, OP.add)
            hif = const.tile([128, 2 * BL], F32, tag=f"ichif{p}")
            nc.vector.tensor_scalar(
                hif[:].bitcast(U32), tu[:], 0xFFFFF000, None, OP.bitwise_and
            )
            hi = const.tile([128, 2 * BL], F32R, tag=f"ichi{p}")
            nc.vector.tensor_copy(hi[:], hif[:])
            ichi.append(hi)
            lo = const.tile([128, 2 * BL], F32R, tag=f"iclo{p}")
            nc.vector.tensor_tensor(lo[:], icf[:], hif[:], OP.subtract)
            iclo.append(lo)

        # ---- state init (pair layout) --------------------------------------
        z = []
        w = []
        ssum = []
        gprev = []
        for p in range(NP):
            tz = state.tile([128, 2 * BL], F32, tag=f"z{p}")
            tw = state.tile([128, 2 * BL], F32, tag=f"w{p}")
            for h in range(2):
                j = 2 * p + h
                jsl = slice(j * 128, (j + 1) * 128)
                hsl = slice(h * BL, (h + 1) * BL)
                nc.sync.dma_start(tz[:, hsl], io["c0T"][jsl, :])
                nc.sync.dma_start(tw[:, hsl], io["v0T"][jsl, :])
            nc.vector.tensor_scalar(tz[:], tz[:], 1.0 / B_SYN, None, OP.mult)
            nc.vector.tensor_scalar(tw[:], tw[:], 1.0 / (B_SYN * B_MEM), None, OP.mult)
            z.append(tz)
            w.append(tw)
            ts_ = state.tile([128, 2 * BL], F32, tag=f"ss{p}")
            nc.gpsimd.memset(ts_[:], 0.0)
            ssum.append(ts_)
            tg = gpool.tile([128, 2 * BL], F32R, tag=f"g{p}")
            nc.vector.tensor_scalar(tg[:], tz[:], 0.0, None, OP.is_gt)
            gprev.append(tg)

        # ---- main recurrence (fully unrolled) ------------------------------
        for _t in range(n_steps):
            g_new = [None] * NP
            for p in range(NP):
                pt = psum.tile([128, 2 * BL], F32, tag="ps")
                # input current preload (identity-stationary f32r matmuls)
                nc.tensor.matmul(pt[:], identr[:], ichi[p][:],
                                 start=True, stop=False, skip_group_check=True)
                nc.tensor.matmul(pt[:], identr[:], iclo[p][:],
                                 start=False, stop=False, skip_group_check=True)
                # recurrent spikes: contract over all 4 k-tiles, hi+lo
                for k in range(KT):
                    gk = gprev[k // 2][:, (k % 2) * BL:(k % 2 + 1) * BL]
                    for h in range(2):
                        j = 2 * p + h
                        jsl = slice(j * 128, (j + 1) * 128)
                        hsl = slice(h * BL, (h + 1) * BL)
                        last = (k == KT - 1) and (h == 1)
                        nc.tensor.matmul(pt[:, hsl], whi[k][:, jsl], gk,
                                         start=False, stop=False,
                                         skip_group_check=True)
                        nc.tensor.matmul(pt[:, hsl], wlo[k][:, jsl], gk,
                                         start=False, stop=last,
                                         skip_group_check=True)
                # z = a_syn*z + (ic + rec)
                nc.vector.scalar_tensor_tensor(
                    z[p][:], z[p][:], A_SYN, pt[:], OP.mult, OP.add
                )
                # next-step spikes
                tg = gpool.tile([128, 2 * BL], F32R, tag=f"g{p}")
                nc.vector.tensor_scalar(tg[:], z[p][:], 0.0, None, OP.is_gt)
                g_new[p] = tg
                # w = a_mem*w + z ; ssum += (w > 0)
                nc.vector.scalar_tensor_tensor(
                    w[p][:], w[p][:], A_MEM, z[p][:], OP.mult, OP.add
                )
                nc.vector.scalar_tensor_tensor(
                    ssum[p][:], w[p][:], 0.0, ssum[p][:], OP.is_gt, OP.add
                )
            gprev = g_new

        # ---- outputs -------------------------------------------------------
        for p in range(NP):
            ro = opool.tile([128, 2 * BL], F32, tag="out")
            nc.vector.tensor_scalar(ro[:], ssum[p][:], 1.0 / n_steps, None, OP.mult)
            co = opool.tile([128, 2 * BL], F32, tag="out")
            nc.vector.tensor_scalar(co[:], z[p][:], B_SYN, None, OP.mult)
            vo = opool.tile([128, 2 * BL], F32, tag="out")
            nc.vector.tensor_scalar(vo[:], w[p][:], B_SYN * B_MEM, None, OP.mult)
            for h in range(2):
                j = 2 * p + h
                jsl = slice(j * 128, (j + 1) * 128)
                hsl = slice(h * BL, (h + 1) * BL)
                nc.sync.dma_start(io["roT"][jsl, :], ro[:, hsl])
                nc.sync.dma_start(io["cT"][jsl, :], co[:, hsl])
                nc.sync.dma_start(io["vT"][jsl, :], vo[:, hsl])


def _build(n_steps: int):
    nc = bacc.Bacc("TRN2", target_bir_lowering=False, debug=False)
    io = {
        "ratesT": nc.dram_tensor("ratesT", [D, BL], F32, kind="ExternalInput").ap(),
        "iwT": nc.dram_tensor("iwT", [D, N], F32, kind="ExternalInput").ap(),
        "w_hi": nc.dram_tensor("w_hi", [N, N], F32R, kind="ExternalInput").ap(),
        "w_lo": nc.dram_tensor("w_lo", [N, N], F32R, kind="ExternalInput").ap(),
        "v0T": nc.dram_tensor("v0T", [N, BL], F32, kind="ExternalInput").ap(),
        "c0T": nc.dram_tensor("c0T", [N, BL], F32, kind="ExternalInput").ap(),
        "roT": nc.dram_tensor("roT", [N, BL], F32, kind="ExternalOutput").ap(),
        "vT": nc.dram_tensor("vT", [N, BL], F32, kind="ExternalOutput").ap(),
        "cT": nc.dram_tensor("cT", [N, BL], F32, kind="ExternalOutput").ap(),
    }
    with tile.TileContext(nc) as tc:
        _lsm_body(tc, n_steps, io)
    nc.compile()
    return nc


_NC_CACHE = {}


def _get_nc(n_steps: int):
    if n_steps not in _NC_CACHE:
        _NC_CACHE[n_steps] = _build(n_steps)
    return _NC_CACHE[n_steps]


def make_in_maps(rates, input_weight, reservoir_weight, v_mem, current):
    rates = np.ascontiguousarray(np.asarray(rates, dtype=np.float32))
    iw = np.ascontiguousarray(np.asarray(input_weight, dtype=np.float32))
    W = np.ascontiguousarray(np.asarray(reservoir_weight, dtype=np.float32))
    v0 = np.ascontiguousarray(np.asarray(v_mem, dtype=np.float32))
    c0 = np.ascontiguousarray(np.asarray(current, dtype=np.float32))

    w_hi = _rne_m11(W)
    w_lo = (W - w_hi).astype(np.float32)
    # deep-denormal residuals (<~1e-42, impossible for real data) are not
    # m11-exact; flushing them loses nothing measurable
    w_lo = np.where(_rne_m11(w_lo) == w_lo, w_lo, np.float32(0.0))
    w_hi = np.ascontiguousarray(w_hi)
    w_lo = np.ascontiguousarray(w_lo)
    iwT = np.ascontiguousarray(iw.T)

    in_maps = []
    for i in range(N_CORES):
        sl = slice(i * BL, (i + 1) * BL)
        in_maps.append({
            "ratesT": np.ascontiguousarray(rates[sl].T),
            "iwT": iwT,
            "w_hi": w_hi,
            "w_lo": w_lo,
            "v0T": np.ascontiguousarray(v0[sl].T),
            "c0T": np.ascontiguousarray(c0[sl].T),
        })
    return in_maps


def assemble(results):
    readout = np.empty((B, N), np.float32)
    v = np.empty((B, N), np.float32)
    c = np.empty((B, N), np.float32)
    for i, res in enumerate(results):
        sl = slice(i * BL, (i + 1) * BL)
        readout[sl] = res["roT"].T
        v[sl] = res["vT"].T
        c[sl] = res["cT"].T
    return readout, (v, c)


def kernel(rates, input_weight, reservoir_weight, v_mem, current, n_steps,
           _trace=False, _trace_kwargs=None):
    n_steps = int(n_steps)
    nc = _get_nc(n_steps)
    in_maps = make_in_maps(rates, input_weight, reservoir_weight, v_mem, current)
    res = run_bass_kernel_spmd(
        nc, in_maps, core_ids=list(range(N_CORES)),
        trace=_trace, **(_trace_kwargs or {}),
    )
    out = assemble(res.results)
    if _trace:
        kernel.last_result = res
    return out


# revision 13
# speedup vs baseline: 3.3892x; 1.0142x over previous
"""Liquid State Machine Bass kernel for Trainium2 (8 NeuronCores, SPMD).

Strategy
--------
Data-parallel over batch: B=2048 is sharded 256/core; input_weight and
reservoir_weight are replicated. All device state is kept TRANSPOSED
(neurons on SBUF partitions, batch on the free dim) so the per-step spike
tensor is directly usable as the matmul moving operand with W tiles as
stationary weights -- no on-device transposes (the host passes
pre-transposed slices, which is free).

The recurrent matmul rec = spikes @ W runs in float32r mode: the PE rounds
operands to e8m11 (measured on HW) but streams at 1 cycle/row vs fp32's 4.
W is Dekker-split on the host into W_hi + W_lo (12 significant bits each,
both exact under the PE's m11 rounding), and spikes are exactly 0/1, so two
f32r matmuls accumulated in fp32 PSUM reproduce the fp32 product.

The per-step input current is folded into the same PSUM accumulation as
two identity-stationary f32r matmuls of an m11-split of ic (split once on
device), which removes a vector-engine op from the inner loop.  State is
rescaled (z = c/beta_syn, w = v/(beta_syn*beta_mem)) so each state update
is a single scalar_tensor_tensor op, and neuron tiles are processed in
pairs (128x512 DVE ops) to amortize per-instruction overhead.  GpSimd is
avoided in the loop (its elementwise ops are ~4us each on HW).
"""

import numpy as np

import concourse.bass as bass
import concourse.mybir as mybir
import concourse.tile as tile
from concourse import bacc
from concourse.bass_utils import run_bass_kernel_spmd
from concourse.masks import make_identity

F32 = mybir.dt.float32
F32R = mybir.dt.float32r
U32 = mybir.dt.uint32
OP = mybir.AluOpType

N_CORES = 8
B, D, N = 2048, 256, 512
BL = B // N_CORES          # batch per core
KT = N // 128              # 4 neuron/contraction tiles
DT = D // 128              # 2 input-dim tiles
NP = KT // 2               # 2 tile-pairs

TAU_MEM = 20.0
TAU_SYN = 5.0
A_MEM = float(np.exp(-1.0 / TAU_MEM))
A_SYN = float(np.exp(-1.0 / TAU_SYN))
B_MEM = 1.0 - A_MEM
B_SYN = 1.0 - A_SYN


def _rne_m11(x: np.ndarray) -> np.ndarray:
    """Round fp32 to 11 mantissa bits, round-to-nearest-even (the PE's
    float32r operand rounding, measured on TRN2 hardware)."""
    xi = x.view(np.uint32)
    r = (xi + np.uint32(0x7FF) + ((xi >> np.uint32(12)) & np.uint32(1))) & np.uint32(
        0xFFFFF000
    )
    return r.view(np.float32)


def _lsm_body(tc, n_steps, io):
    nc = tc.nc
    with tc.tile_pool(name="const", bufs=1) as const, \
         tc.tile_pool(name="state", bufs=1) as state, \
         tc.tile_pool(name="g", bufs=3) as gpool, \
         tc.tile_pool(name="outs", bufs=4) as opool, \
         tc.tile_pool(name="ps", bufs=6, space="PSUM") as psum:

        # ---- load constants ------------------------------------------------
        # small inputs first so the ic pipeline starts while W streams in
        iwt = []
        rt = []
        for d in range(DT):
            ti = const.tile([128, N], F32, tag=f"iwt{d}")
            nc.sync.dma_start(ti[:], io["iwT"][d * 128:(d + 1) * 128, :])
            iwt.append(ti)
            tr = const.tile([128, BL], F32, tag=f"ratesT{d}")
            nc.sync.dma_start(tr[:], io["ratesT"][d * 128:(d + 1) * 128, :])
            rt.append(tr)

        whi = []
        wlo = []
        for k in range(KT):
            th = const.tile([128, N], F32R, tag=f"whi{k}")
            nc.sync.dma_start(th[:], io["w_hi"][k * 128:(k + 1) * 128, :])
            whi.append(th)
            tl = const.tile([128, N], F32R, tag=f"wlo{k}")
            nc.sync.dma_start(tl[:], io["w_lo"][k * 128:(k + 1) * 128, :])
            wlo.append(tl)

        ident = const.tile([128, 128], F32, tag="identf")
        make_identity(nc, ident[:])
        identr = const.tile([128, 128], F32R, tag="identr")
        nc.vector.tensor_copy(identr[:], ident[:])

        # ---- input current: icT = iw @ rates.T (true fp32), m11-split ------
        # pair p holds n-tiles (2p, 2p+1) side by side: [128, 512]
        ichi = []
        iclo = []
        for p in range(NP):
            pt = psum.tile([128, 2 * BL], F32, tag="ps")
            for h in range(2):
                j = 2 * p + h
                jsl = slice(j * 128, (j + 1) * 128)
                hsl = slice(h * BL, (h + 1) * BL)
                for d in range(DT):
                    nc.tensor.matmul(
                        pt[:, hsl], iwt[d][:, jsl], rt[d][:],
                        start=(d == 0), stop=(d == DT - 1),
                        skip_group_check=True,
                    )
            icf = const.tile([128, 2 * BL], F32, tag=f"icf{p}")
            nc.vector.tensor_copy(icf[:], pt[:])
            # m11 Dekker split on device (uint32 bit arithmetic)
            icu = icf[:].bitcast(U32)
            tu = const.tile([128, 2 * BL], U32, tag=f"ictmp{p}")
            nc.vector.tensor_scalar(tu[:], icu, 12, None, OP.logical_shift_right)
            nc.vector.tensor_scalar(tu[:], tu[:], 1, None, OP.bitwise_and)
            nc.vector.scalar_tensor_tensor(tu[:], tu[:], 0x7FF, icu, OP.add, OP.add)
            hif = const.tile([128, 2 * BL], F32, tag=f"ichif{p}")
            nc.vector.tensor_scalar(
                hif[:].bitcast(U32), tu[:], 0xFFFFF000, None, OP.bitwise_and
            )
            hi = const.tile([128, 2 * BL], F32R, tag=f"ichi{p}")
            nc.vector.tensor_copy(hi[:], hif[:])
            ichi.append(hi)
            lo = const.tile([128, 2 * BL], F32R, tag=f"iclo{p}")
            nc.vector.tensor_tensor(lo[:], icf[:], hif[:], OP.subtract)
            iclo.append(lo)

        # ---- state init (pair layout) --------------------------------------
        z = []
        w = []
        ssum = []
        gprev = []
        for p in range(NP):
            tz = state.tile([128, 2 * BL], F32, tag=f"z{p}")
            tw = state.tile([128, 2 * BL], F32, tag=f"w{p}")
            for h in range(2):
                j = 2 * p + h
                jsl = slice(j * 128, (j + 1) * 128)
                hsl = slice(h * BL, (h + 1) * BL)
                nc.sync.dma_start(tz[:, hsl], io["c0T"][jsl, :])
                nc.sync.dma_start(tw[:, hsl], io["v0T"][jsl, :])
            nc.vector.tensor_scalar(tz[:], tz[:], 1.0 / B_SYN, None, OP.mult)
            nc.vector.tensor_scalar(tw[:], tw[:], 1.0 / (B_SYN * B_MEM), None, OP.mult)
            z.append(tz)
            w.append(tw)
            ts_ = state.tile([128, 2 * BL], F32, tag=f"ss{p}")
            nc.gpsimd.memset(ts_[:], 0.0)
            ssum.append(ts_)
            tg = gpool.tile([128, 2 * BL], F32R, tag=f"g{p}")
            nc.vector.tensor_scalar(tg[:], tz[:], 0.0, None, OP.is_gt)
            gprev.append(tg)

        # ---- main recurrence (fully unrolled) ------------------------------
        for _t in range(n_steps):
            g_new = [None] * NP
            for p in range(NP):
                pt = psum.tile([128, 2 * BL], F32, tag="ps")
                # input current preload (identity-stationary f32r matmuls)
                nc.tensor.matmul(pt[:], identr[:], ichi[p][:],
                                 start=True, stop=False, skip_group_check=True)
                nc.tensor.matmul(pt[:], identr[:], iclo[p][:],
                                 start=False, stop=False, skip_group_check=True)
                # recurrent spikes: contract over all 4 k-tiles, hi+lo
                for k in range(KT):
                    gk = gprev[k // 2][:, (k % 2) * BL:(k % 2 + 1) * BL]
                    for h in range(2):
                        j = 2 * p + h
                        jsl = slice(j * 128, (j + 1) * 128)
                        hsl = slice(h * BL, (h + 1) * BL)
                        last = (k == KT - 1) and (h == 1)
                        nc.tensor.matmul(pt[:, hsl], whi[k][:, jsl], gk,
                                         start=False, stop=False,
                                         skip_group_check=True)
                        nc.tensor.matmul(pt[:, hsl], wlo[k][:, jsl], gk,
                                         start=False, stop=last,
                                         skip_group_check=True)
                # z = a_syn*z + (ic + rec)
                nc.vector.scalar_tensor_tensor(
                    z[p][:], z[p][:], A_SYN, pt[:], OP.mult, OP.add
                )
                # next-step spikes
                tg = gpool.tile([128, 2 * BL], F32R, tag=f"g{p}")
                nc.vector.tensor_scalar(tg[:], z[p][:], 0.0, None, OP.is_gt)
                g_new[p] = tg
                # w = a_mem*w + z ; ssum += (w > 0)
                nc.vector.scalar_tensor_tensor(
                    w[p][:], w[p][:], A_MEM, z[p][:], OP.mult, OP.add
                )
                nc.vector.scalar_tensor_tensor(
                    ssum[p][:], w[p][:], 0.0, ssum[p][:], OP.is_gt, OP.add
                )
            gprev = g_new

        # ---- outputs -------------------------------------------------------
        for p in range(NP):
            ro = opool.tile([128, 2 * BL], F32, tag="out")
            nc.vector.tensor_scalar(ro[:], ssum[p][:], 1.0 / n_steps, None, OP.mult)
            co = opool.tile([128, 2 * BL], F32, tag="out")
            nc.vector.tensor_scalar(co[:], z[p][:], B_SYN, None, OP.mult)
            vo = opool.tile([128, 2 * BL], F32, tag="out")
            nc.vector.tensor_scalar(vo[:], w[p][:], B_SYN * B_MEM, None, OP.mult)
            for h in range(2):
                j = 2 * p + h
                jsl = slice(j * 128, (j + 1) * 128)
                hsl = slice(h * BL, (h + 1) * BL)
                nc.sync.dma_start(io["roT"][jsl, :], ro[:, hsl])
                nc.sync.dma_start(io["cT"][jsl, :], co[:, hsl])
                nc.sync.dma_start(io["vT"][jsl, :], vo[:, hsl])


def _build(n_steps: int):
    nc = bacc.Bacc("TRN2", target_bir_lowering=False, debug=False)
    io = {
        "ratesT": nc.dram_tensor("ratesT", [D, BL], F32, kind="ExternalInput").ap(),
        "iwT": nc.dram_tensor("iwT", [D, N], F32, kind="ExternalInput").ap(),
        "w_hi": nc.dram_tensor("w_hi", [N, N], F32R, kind="ExternalInput").ap(),
        "w_lo": nc.dram_tensor("w_lo", [N, N], F32R, kind="ExternalInput").ap(),
        "v0T": nc.dram_tensor("v0T", [N, BL], F32, kind="ExternalInput").ap(),
        "c0T": nc.dram_tensor("c0T", [N, BL], F32, kind="ExternalInput").ap(),
        "roT": nc.dram_tensor("roT", [N, BL], F32, kind="ExternalOutput").ap(),
        "vT": nc.dram_tensor("vT", [N, BL], F32, kind="ExternalOutput").ap(),
        "cT": nc.dram_tensor("cT", [N, BL], F32, kind="ExternalOutput").ap(),
    }
    with tile.TileContext(nc) as tc:
        _lsm_body(tc, n_steps, io)
    nc.compile()
    return nc


_NC_CACHE = {}


def _get_nc(n_steps: int):
    if n_steps not in _NC_CACHE:
        _NC_CACHE[n_steps] = _build(n_steps)
    return _NC_CACHE[n_steps]


def make_in_maps(rates, input_weight, reservoir_weight, v_mem, current):
    rates = np.ascontiguousarray(np.asarray(rates, dtype=np.float32))
    iw = np.ascontiguousarray(np.asarray(input_weight, dtype=np.float32))
    W = np.ascontiguousarray(np.asarray(reservoir_weight, dtype=np.float32))
    v0 = np.ascontiguousarray(np.asarray(v_mem, dtype=np.float32))
    c0 = np.ascontiguousarray(np.asarray(current, dtype=np.float32))

    w_hi = _rne_m11(W)
    w_lo = (W - w_hi).astype(np.float32)
    # deep-denormal residuals (<~1e-42, impossible for real data) are not
    # m11-exact; flushing them loses nothing measurable
    w_lo = np.where(_rne_m11(w_lo) == w_lo, w_lo, np.float32(0.0))
    w_hi = np.ascontiguousarray(w_hi)
    w_lo = np.ascontiguousarray(w_lo)
    iwT = np.ascontiguousarray(iw.T)

    in_maps = []
    for i in range(N_CORES):
        sl = slice(i * BL, (i + 1) * BL)
        in_maps.append({
            "ratesT": np.ascontiguousarray(rates[sl].T),
            "iwT": iwT,
            "w_hi": w_hi,
            "w_lo": w_lo,
            "v0T": np.ascontiguousarray(v0[sl].T),
            "c0T": np.ascontiguousarray(c0[sl].T),
        })
    return in_maps


def assemble(results):
    readout = np.empty((B, N), np.float32)
    v = np.empty((B, N), np.float32)
    c = np.empty((B, N), np.float32)
    for i, res in enumerate(results):
        sl = slice(i * BL, (i + 1) * BL)
        readout[sl] = res["roT"].T
        v[sl] = res["vT"].T
        c[sl] = res["cT"].T
    return readout, (v, c)


def kernel(rates, input_weight, reservoir_weight, v_mem, current, n_steps,
           _trace=False, _trace_kwargs=None):
    n_steps = int(n_steps)
    nc = _get_nc(n_steps)
    in_maps = make_in_maps(rates, input_weight, reservoir_weight, v_mem, current)
    res = run_bass_kernel_spmd(
        nc, in_maps, core_ids=list(range(N_CORES)),
        trace=_trace, **(_trace_kwargs or {}),
    )
    out = assemble(res.results)
    if _trace:
        kernel.last_result = res
    return out
